# revision 1
# baseline (speedup 1.0000x reference)
"""DeepGAT (4-layer GAT + BN + residual + MLP head) on 8 Trainium2 cores.

Sharding: nodes are dst-partitioned across the 8 cores (1250 nodes/core).
Edges are routed on the host to the core owning their dst node and sorted by
dst. Weights are replicated. Per layer each core projects all N nodes
(replicated compute, cheap on PE), writes per-node rows
[xl bf16 x1024 | alpha_src bf16 x8 | pad] to its local DRAM, then gathers the
rows of its edges' src nodes with dma_gather. Segment softmax is computed
with the normalization applied AFTER aggregation (mathematically identical),
so the edge phase is a single pass. h is exchanged with an AllGather.
"""

import numpy as np

import concourse.bass as bass
import concourse.bacc as bacc
import concourse.mybir as mybir
from concourse.tile import TileContext
from concourse.tile_rust import add_dep_helper

FP32 = mybir.dt.float32
BF16 = mybir.dt.bfloat16
I16 = mybir.dt.int16
AF = mybir.ActivationFunctionType
OP = mybir.AluOpType

# problem constants (hardcoded per harness contract)
ALPHA = 0.1
BN_EPS = 1e-5
NEG_SLOPE = 0.2
NEG_BIG = -30000.0
HID = 128  # partition width; fixed


def _set_dims(n=10000, e=160000, in_dim=512, heads=8, layers=4, cls=2, cores=8):
    """Set problem dims as module globals (parametrized for sim tests)."""
    g = globals()
    g["N"], g["E"], g["IN"], g["H"], g["L"], g["CLS"], g["M"] = (
        n, e, in_dim, heads, layers, cls, cores)
    g["NPC"] = n // cores
    g["NPC_PAD"] = -(-g["NPC"] // 128) * 128
    g["NBLK"] = g["NPC_PAD"] // 128
    g["N_PAD"] = -(-n // 128) * 128
    g["NNB"] = g["N_PAD"] // 128
    g["HC"] = heads * HID
    g["PRJ"] = g["HC"] + 2 * heads
    g["ROW"] = -(-(g["HC"] + heads) // 128) * 128
    g["AROW"] = 64


_set_dims()


class Cfg:
    """Static schedule computed from the actual edge data."""

    def __init__(self, chunks_per_block):
        self.chunks_per_block = list(chunks_per_block)
        self.CH = sum(self.chunks_per_block)
        self.TOTE = 128 * self.CH


def _pack_idx16(idx, pad_to=None):
    """Pack int16 indices for dma_gather: idx i at [i%16, i//16], replicated
    to 128 partitions."""
    idx = np.asarray(idx, np.int64)
    n = len(idx)
    if pad_to is not None:
        assert pad_to >= n
        idx = np.concatenate([idx, np.zeros(pad_to - n, np.int64)])
        n = pad_to
    assert n % 16 == 0
    a = idx.astype(np.int16).reshape(n // 16, 16).T  # [16, n//16]
    return np.tile(a, (8, 1)).copy()  # [128, n//16]


def preprocess(x, edge_index, Wp, bp, Wl, att_src, att_dst, bl, gamma, beta,
               W1, b1, W2, b2):
    """Host-side: edge routing/sorting per core + weight folding."""
    x = np.asarray(x, np.float32)
    src = np.concatenate([np.asarray(edge_index[0]), np.arange(N)]).astype(np.int64)
    dst = np.concatenate([np.asarray(edge_index[1]), np.arange(N)]).astype(np.int64)

    per_core = []
    for k in range(M):
        m = (dst // NPC) == k
        s_k, d_k = src[m], dst[m] - k * NPC
        order = np.argsort(d_k, kind="stable")
        per_core.append((s_k[order], d_k[order]))

    counts = np.zeros((M, NBLK), np.int64)
    for k in range(M):
        _, d_k = per_core[k]
        b = d_k // 128
        for bb in range(NBLK):
            counts[k, bb] = int((b == bb).sum())
    chunks_per_block = [max(1, int(np.ceil(counts[:, bb].max() / 128)))
                        for bb in range(NBLK)]
    cfg = Cfg(chunks_per_block)

    per_core_inputs = []
    for k in range(M):
        s_k, d_k = per_core[k]
        b_k = d_k // 128
        srcidx = np.zeros(cfg.TOTE, np.int64)
        dstidx = np.zeros(cfg.TOTE, np.int64)
        dstloc = np.zeros(cfg.TOTE, np.int64)
        padb = np.full(cfg.TOTE, NEG_BIG, np.float32)
        off = 0
        for bb in range(NBLK):
            sel = b_k == bb
            cnt = int(sel.sum())
            cap = 128 * cfg.chunks_per_block[bb]
            assert cnt <= cap, (k, bb, cnt, cap)
            srcidx[off:off + cnt] = s_k[sel]
            dstidx[off:off + cnt] = d_k[sel] + k * NPC
            dstloc[off:off + cnt] = d_k[sel] - 128 * bb
            padb[off:off + cnt] = 0.0
            off += cap
        assert off == cfg.TOTE

        xT_own = np.zeros((IN, NPC_PAD), np.float32)
        xT_own[:, :NPC] = x[k * NPC:(k + 1) * NPC].T

        per_core_inputs.append({
            "srcidx": _pack_idx16(srcidx),
            "dstidx": _pack_idx16(dstidx),
            "dstloc": dstloc.reshape(cfg.CH, 128).T.astype(np.float32).copy(),
            "padbias": padb.reshape(cfg.CH, 128).T.copy(),
            "xT_own": xT_own,
        })

    # weight folding
    Wl = np.asarray(Wl, np.float32)          # [L, HID, HC]
    a_s = np.asarray(att_src, np.float32)    # [L, H, HID]
    a_d = np.asarray(att_dst, np.float32)
    Wcat = np.zeros((L, HID, PRJ), np.float32)
    for i in range(L):
        Wcat[i, :, :HC] = Wl[i]
        w3 = Wl[i].reshape(HID, H, HID)
        Wcat[i, :, HC:HC + H] = np.einsum("khc,hc->kh", w3, a_s[i])
        Wcat[i, :, HC + H:] = np.einsum("khc,hc->kh", w3, a_d[i])

    bn_inv = 1.0 / np.sqrt(1.0 + BN_EPS)
    gamma = np.asarray(gamma, np.float32)
    beta = np.asarray(beta, np.float32)
    bl = np.asarray(bl, np.float32)
    # h = elu((1-a)*(gamma*bn_inv*(mean+bl)+beta) + a*prev); fold 1/H into s.
    s_aff = ((1.0 - ALPHA) * gamma * bn_inv / H).T.copy()            # [HID, L]
    t_aff = ((1.0 - ALPHA) * (gamma * bn_inv * bl + beta)).T.copy()  # [HID, L]

    iota_sq = np.broadcast_to(np.arange(128, dtype=np.float32), (128, 128)).copy()
    ident = np.eye(128, dtype=np.float32)

    shared = {
        "Wp": np.asarray(Wp, np.float32),
        "bp": np.asarray(bp, np.float32)[:, None],
        "Wcat": Wcat,
        "s_aff": s_aff, "t_aff": t_aff,
        "W1": np.asarray(W1, np.float32),
        "b1": np.asarray(b1, np.float32)[:, None],
        "W2": np.asarray(W2, np.float32),
        "b2": np.asarray(b2, np.float32)[:, None],
        "iota_sq": iota_sq, "ident": ident,
    }
    return cfg, shared, per_core_inputs


def _elu(nc, p, out_ap, z_ap, shape, tg):
    """out = elu(z) = relu(z) + exp(min(z,0)) - 1, z in SBUF f32."""
    P, F = shape
    mn = p.tile([P, F], FP32, tag=f"elu_mn_{tg}")
    ex = p.tile([P, F], FP32, tag=f"elu_ex_{tg}")
    rl = p.tile([P, F], FP32, tag=f"elu_rl_{tg}")
    nc.vector.tensor_scalar_min(out=mn[:], in0=z_ap, scalar1=0.0)
    nc.scalar.activation(out=ex[:], in_=mn[:], func=AF.Exp)
    nc.vector.tensor_scalar_max(out=rl[:], in0=z_ap, scalar1=0.0)
    nc.vector.tensor_tensor(out=rl[:], in0=rl[:], in1=ex[:], op=OP.add)
    nc.vector.tensor_scalar_sub(out=out_ap, in0=rl[:], scalar1=1.0)


def build(nc, cfg):
    """Emit the SPMD program (dims from module globals)."""
    n, n_pad, npc, npc_pad = N, N_PAD, NPC, NPC_PAD
    in_dim, layers, heads, cores = IN, L, H, M
    nblk, nnb, hc, prj = NBLK, NNB, HC, PRJ
    qd = HID // 2

    # ---------------- I/O ----------------
    srcidx = nc.dram_tensor("srcidx", [128, cfg.TOTE // 16], I16, kind="ExternalInput")
    dstidx = nc.dram_tensor("dstidx", [128, cfg.TOTE // 16], I16, kind="ExternalInput")
    dstloc_in = nc.dram_tensor("dstloc", [128, cfg.CH], FP32, kind="ExternalInput")
    padbias_in = nc.dram_tensor("padbias", [128, cfg.CH], FP32, kind="ExternalInput")
    xT_own_in = nc.dram_tensor("xT_own", [in_dim, npc_pad], FP32, kind="ExternalInput")
    Wp_in = nc.dram_tensor("Wp", [in_dim, HID], FP32, kind="ExternalInput")
    bp_in = nc.dram_tensor("bp", [HID, 1], FP32, kind="ExternalInput")
    Wcat_in = nc.dram_tensor("Wcat", [layers, HID, prj], FP32, kind="ExternalInput")
    s_aff_in = nc.dram_tensor("s_aff", [HID, layers], FP32, kind="ExternalInput")
    t_aff_in = nc.dram_tensor("t_aff", [HID, layers], FP32, kind="ExternalInput")
    W1_in = nc.dram_tensor("W1", [HID, qd], FP32, kind="ExternalInput")
    b1_in = nc.dram_tensor("b1", [qd, 1], FP32, kind="ExternalInput")
    W2_in = nc.dram_tensor("W2", [qd, CLS], FP32, kind="ExternalInput")
    b2_in = nc.dram_tensor("b2", [CLS, 1], FP32, kind="ExternalInput")
    iota_in = nc.dram_tensor("iota_sq", [128, 128], FP32, kind="ExternalInput")
    ident_in = nc.dram_tensor("ident", [128, 128], FP32, kind="ExternalInput")
    out_dram = nc.dram_tensor("out", [CLS, npc_pad], FP32, kind="ExternalOutput")

    agout = nc.dram_tensor("h_agout", [cores * HID, npc_pad], FP32,
                           addr_space="Shared" if cores > 4 else "Local")

    with TileContext(nc) as tc:
        with (
            tc.tile_pool(name="const", bufs=1) as cpool,
            tc.tile_pool(name="hbuf", bufs=1) as hpool,
            tc.tile_pool(name="proj", bufs=2) as ppool,
            tc.tile_pool(name="gath", bufs=3) as gpool,
            tc.tile_pool(name="edge", bufs=3) as epool,
            tc.tile_pool(name="blk", bufs=2) as bpool,
            tc.tile_pool(name="wide", bufs=1) as wpool,
            tc.tile_pool(name="dram", bufs=1, space="DRAM") as dpool,
            tc.tile_pool(name="psA", bufs=1, space="PSUM") as psA,
            tc.tile_pool(name="psS", bufs=3, space="PSUM") as psS,
        ):
            # dma_gather allocates a register per distinct count; cache them
            _regs = {}

            def nreg(v):
                if v not in _regs:
                    _regs[v] = nc.gpsimd.to_reg(v)
                return _regs[v]

            # ---------------- resident constants / state ----------------
            iota_bf = cpool.tile([128, 128], BF16)
            nc.gpsimd.dma_start(out=iota_bf[:], in_=iota_in[:, :])
            ident_f = cpool.tile([128, 128], FP32)
            nc.sync.dma_start(out=ident_f[:], in_=ident_in[:, :])
            dstloc_f = cpool.tile([128, cfg.CH], FP32)
            nc.sync.dma_start(out=dstloc_f[:], in_=dstloc_in[:, :])
            srcidx_sb = cpool.tile([128, cfg.TOTE // 16], I16)
            nc.sync.dma_start(out=srcidx_sb[:], in_=srcidx[:, :])
            padbias = cpool.tile([128, cfg.CH], FP32)
            nc.sync.dma_start(out=padbias[:], in_=padbias_in[:, :])
            dstidx_sb = cpool.tile([128, cfg.TOTE // 16], I16)
            nc.sync.dma_start(out=dstidx_sb[:], in_=dstidx[:, :])
            s_aff = cpool.tile([128, layers], FP32)
            nc.sync.dma_start(out=s_aff[:], in_=s_aff_in[:, :])
            t_aff = cpool.tile([128, layers], FP32)
            nc.sync.dma_start(out=t_aff[:], in_=t_aff_in[:, :])
            W1_sb = cpool.tile([128, qd], FP32)
            nc.sync.dma_start(out=W1_sb[:], in_=W1_in[:, :])
            b1_sb = cpool.tile([qd, 1], FP32)
            nc.sync.dma_start(out=b1_sb[:], in_=b1_in[:, :])
            W2_sb = cpool.tile([qd, CLS], FP32)
            nc.sync.dma_start(out=W2_sb[:], in_=W2_in[:, :])
            b2_sb = cpool.tile([CLS, 1], FP32)
            nc.sync.dma_start(out=b2_sb[:], in_=b2_in[:, :])
            bp_sb = cpool.tile([HID, 1], FP32)
            nc.sync.dma_start(out=bp_sb[:], in_=bp_in[:, :])

            hT = hpool.tile([128, n_pad], FP32, tag="hT")
            if n_pad > n:
                nc.vector.memset(hT[:, n:], 0.0)
            h_own = [hpool.tile([128, npc_pad], FP32, tag=f"h_own{i}",
                                name=f"h_own{i}")
                     for i in range(2)]

            kchunks = in_dim // 128

            # ------- h0 = elu(x @ Wp + bp), own nodes only (scoped pool) ----
            with tc.tile_pool(name="x0", bufs=2) as x0pool:
                Wp_sb = cpool.tile([128, kchunks, HID], FP32)
                for kc in range(kchunks):
                    nc.sync.dma_start(out=Wp_sb[:, kc, :],
                                      in_=Wp_in[kc * 128:(kc + 1) * 128, :])
                z0 = wpool.tile([128, npc_pad], FP32, tag="zw")
                pieces0 = [(j0, min(j0 + 512, npc_pad))
                           for j0 in range(0, npc_pad, 512)]
                h0_pss = [psS.tile([128, j1 - j0], FP32, tag="mm512",
                                   name=f"h0_ps{j0}") for j0, j1 in pieces0]
                for kc in range(kchunks):
                    xt = x0pool.tile([128, npc_pad], FP32, tag="xT",
                                     name=f"xT{kc}")
                    nc.sync.dma_start(out=xt[:],
                                      in_=xT_own_in[kc * 128:(kc + 1) * 128, :])
                    for (j0, j1), ps in zip(pieces0, h0_pss):
                        nc.tensor.matmul(out=ps[:],
                                         lhsT=Wp_sb[:, kc, :],
                                         rhs=xt[:, j0:j1],
                                         start=(kc == 0),
                                         stop=(kc == kchunks - 1))
                for (j0, j1), ps in zip(pieces0, h0_pss):
                    nc.scalar.activation(out=z0[:, j0:j1], in_=ps[:],
                                         func=AF.Identity,
                                         bias=bp_sb[:, :1], scale=1.0)
                _elu(nc, wpool, h_own[0][:], z0[:], (128, npc_pad), "w")

            # ---------------- layers ----------------
            for li in range(layers):
                hprev = h_own[li % 2]
                hnew = h_own[(li + 1) % 2]

                # --- allgather h (own cols -> full hT) ---
                bounce = dpool.tile([HID, npc_pad], FP32, tag="bounce")
                nc.sync.dma_start(out=bounce[:], in_=hprev[:])
                cc = nc.gpsimd.collective_compute(
                    "AllGather", OP.bypass,
                    replica_groups=[list(range(cores))],
                    ins=[bounce[:]], outs=[agout[:, :]],
                )
                for k in range(cores):
                    d = nc.sync.dma_start(
                        out=hT[:, k * npc:(k + 1) * npc],
                        in_=agout[k * HID:(k + 1) * HID, :npc])
                    add_dep_helper(d.ins, cc.ins, True, "cc->readback")

                # --- projection: all nodes, xl | alpha_s | alpha_d ---
                Wc = ppool.tile([128, prj], FP32, tag="Wc")
                nc.sync.dma_start(out=Wc[:], in_=Wcat_in[li, :, :])
                xlrow_t = dpool.tile([n_pad, ROW], BF16, tag="xlrow")
                ad_t = dpool.tile([n_pad, AROW], FP32, tag="adrow")
                tbl_writes = []
                for nb in range(nnb):
                    xlwr = ppool.tile([128, ROW], BF16, tag="xlwr")
                    adwr = ppool.tile([128, AROW], FP32, tag="adwr")
                    nc.vector.memset(adwr[:, heads:], 0.0)
                    if ROW > hc + heads:
                        nc.vector.memset(xlwr[:, hc + heads:], 0.0)
                    pieces = [(j0, min(j0 + 512, hc))
                              for j0 in range(0, hc, 512)] + [(hc, prj)]
                    for j0, j1 in pieces:
                        pp = psS.tile([128, j1 - j0], FP32, tag="mm512",
                                      name="pp")
                        nc.tensor.matmul(out=pp[:],
                                         lhsT=hT[:, nb * 128:(nb + 1) * 128],
                                         rhs=Wc[:, j0:j1],
                                         start=True, stop=True)
                        if j1 <= hc:
                            nc.scalar.activation(out=xlwr[:, j0:j1], in_=pp[:],
                                                 func=AF.Copy)
                        else:
                            nc.scalar.activation(out=xlwr[:, hc:hc + heads],
                                                 in_=pp[:, :heads], func=AF.Copy)
                            nc.vector.tensor_copy(out=adwr[:, :heads],
                                                  in_=pp[:, heads:2 * heads])
                    w1_ = nc.sync.dma_start(
                        out=xlrow_t[nb * 128:(nb + 1) * 128, :], in_=xlwr[:])
                    w2_ = nc.sync.dma_start(
                        out=ad_t[nb * 128:(nb + 1) * 128, :], in_=adwr[:])
                    tbl_writes += [w1_, w2_]

                # --- edge phase, per dst block ---
                off = 0
                for bb in range(nblk):
                    cb = cfg.chunks_per_block[bb]
                    # even split so chunk pairs never straddle the halves
                    c1 = min(cb, 2 * ((cb + 3) // 4))
                    gts, gtds, splits = [], [], []
                    for (lo, sz) in ((0, c1), (c1, cb - c1)):
                        if sz == 0:
                            continue
                        g = gpool.tile([128, sz, ROW], BF16, tag="gt")
                        g1_ = nc.gpsimd.dma_gather(
                            out_ap=g[:], in_ap=xlrow_t[:],
                            idxs_ap=srcidx_sb[:, (off + lo) * 8:(off + lo + sz) * 8],
                            num_idxs=128 * sz, num_idxs_reg=nreg(128 * sz),
                            elem_size=ROW, single_packet=128 * sz <= 1024)
                        gd = gpool.tile([128, sz, AROW], FP32, tag="gtd")
                        g2_ = nc.gpsimd.dma_gather(
                            out_ap=gd[:], in_ap=ad_t[:],
                            idxs_ap=dstidx_sb[:, (off + lo) * 8:(off + lo + sz) * 8],
                            num_idxs=128 * sz, num_idxs_reg=nreg(128 * sz),
                            elem_size=AROW, single_packet=128 * sz <= 1024)
                        for w_ in tbl_writes:
                            add_dep_helper(g1_.ins, w_.ins, True, "tbl->gather")
                            add_dep_helper(g2_.ins, w_.ins, True, "tbl->gather")
                        gts.append(g)
                        gtds.append(gd)
                        splits.append((lo, sz))

                    agg = psA.tile([128, hc], FP32, tag="agg")
                    den = psS.tile([128, heads], FP32, tag="small", name="den")

                    for j in range(cb):
                        c = off + j
                        gi = 0 if j < c1 else 1
                        gt, gtd = gts[gi], gtds[gi]
                        jj = j - splits[gi][0]
                        # one-hot [e,d] via iota == dstloc
                        oh = epool.tile([128, 128], BF16, tag="oh")
                        nc.vector.tensor_scalar(
                            out=oh[:], in0=iota_bf[:],
                            scalar1=dstloc_f[:, c:c + 1], scalar2=None,
                            op0=OP.is_equal)
                        # s = as + ad ; p = exp(lrelu(s)) bf16 (pads: -3e4)
                        sv = epool.tile([128, heads], FP32, tag="sv")
                        nc.vector.tensor_tensor(
                            out=sv[:], in0=gt[:, jj, hc:hc + heads],
                            in1=gtd[:, jj, :heads], op=OP.add)
                        # lrelu(x) = max(x, slope*x) for 0<slope<1
                        lr = epool.tile([128, heads], FP32, tag="lr")
                        nc.vector.tensor_scalar_mul(out=lr[:], in0=sv[:],
                                                    scalar1=NEG_SLOPE)
                        nc.vector.tensor_tensor(out=lr[:], in0=sv[:],
                                                in1=lr[:], op=OP.max)
                        pe = epool.tile([128, heads, 1], BF16, tag="pe")
                        nc.scalar.activation(
                            out=pe[:].rearrange("p a b -> p (a b)"), in_=lr[:],
                            func=AF.Exp, bias=padbias[:, c:c + 1])
                        # msg = gathered * p (p broadcast across channels)
                        msg = epool.tile([128, hc], BF16, tag="msg")
                        nc.vector.tensor_tensor(
                            out=msg[:].rearrange("p (a b) -> p a b", a=heads),
                            in0=gt[:, jj, :hc].rearrange(
                                "p (a b) -> p a b", a=heads),
                            in1=pe[:].to_broadcast([128, heads, HID]),
                            op=OP.mult)
                        first, last = j == 0, j == cb - 1
                        nc.tensor.matmul(out=den[:], lhsT=oh[:],
                                         rhs=pe[:].rearrange("p a b -> p (a b)"),
                                         start=first, stop=last,
                                         skip_group_check=True)
                        for j0 in range(0, hc, 512):
                            j1 = min(j0 + 512, hc)
                            nc.tensor.matmul(out=agg[:, j0:j1], lhsT=oh[:],
                                             rhs=msg[:, j0:j1],
                                             start=first, stop=last,
                                             skip_group_check=True)
                    off += cb

                    # --- block epilogue ---
                    rec = bpool.tile([128, heads], FP32, tag="rec")
                    # clamp: pad dst lanes have denom 0 (reciprocal of 0
                    # faults / yields inf); any tiny floor works, their
                    # output is discarded
                    nc.vector.tensor_scalar_max(out=rec[:], in0=den[:],
                                                scalar1=1e-20)
                    nc.vector.reciprocal(out=rec[:], in_=rec[:])
                    hm = bpool.tile([128, HID], FP32, tag="hm")
                    tmp = bpool.tile([128, HID], FP32, tag="hmt")
                    for hh in range(heads):
                        dst_t = hm if hh == 0 else tmp
                        nc.scalar.activation(
                            out=dst_t[:], in_=agg[:, hh * HID:(hh + 1) * HID],
                            func=AF.Copy, scale=rec[:, hh:hh + 1])
                        if hh > 0:
                            nc.vector.tensor_tensor(out=hm[:], in0=hm[:],
                                                    in1=tmp[:], op=OP.add)
                    mT_ps = psS.tile([128, 128], FP32, tag="small",
                                     name="mT_ps")
                    nc.tensor.transpose(out=mT_ps[:], in_=hm[:],
                                        identity=ident_f[:])
                    z1 = bpool.tile([128, 128], FP32, tag="z1")
                    nc.scalar.activation(out=z1[:], in_=mT_ps[:], func=AF.Identity,
                                         bias=t_aff[:, li:li + 1],
                                         scale=s_aff[:, li:li + 1])
                    z2 = bpool.tile([128, 128], FP32, tag="z2")
                    nc.vector.tensor_scalar_mul(
                        out=z2[:], in0=hprev[:, bb * 128:(bb + 1) * 128],
                        scalar1=ALPHA)
                    nc.vector.tensor_tensor(out=z1[:], in0=z1[:], in1=z2[:],
                                            op=OP.add)
                    _elu(nc, bpool, hnew[:, bb * 128:(bb + 1) * 128], z1[:],
                         (128, 128), "n")

            # ---------------- classifier ----------------
            hfin = h_own[layers % 2]
            zc = wpool.tile([qd, npc_pad], FP32, tag="zw")
            for j0 in range(0, npc_pad, 512):
                j1 = min(j0 + 512, npc_pad)
                hid_ps = psS.tile([qd, j1 - j0], FP32, tag="mm512",
                                  name="hid_ps")
                nc.tensor.matmul(out=hid_ps[:], lhsT=W1_sb[:],
                                 rhs=hfin[:, j0:j1], start=True, stop=True)
                nc.scalar.activation(out=zc[:, j0:j1], in_=hid_ps[:],
                                     func=AF.Identity,
                                     bias=b1_sb[:, :1], scale=1.0)
            hidsb = wpool.tile([qd, npc_pad], FP32, tag="hidsb")
            _elu(nc, wpool, hidsb[:], zc[:], (qd, npc_pad), "w")
            osb = wpool.tile([CLS, npc_pad], FP32, tag="osb")
            for j0 in range(0, npc_pad, 512):
                j1 = min(j0 + 512, npc_pad)
                out_ps = psS.tile([CLS, j1 - j0], FP32, tag="mm512",
                                  name="out_ps")
                nc.tensor.matmul(out=out_ps[:], lhsT=W2_sb[:],
                                 rhs=hidsb[:, j0:j1], start=True, stop=True)
                nc.scalar.activation(out=osb[:, j0:j1], in_=out_ps[:],
                                     func=AF.Identity,
                                     bias=b2_sb[:, :1], scale=1.0)
            nc.sync.dma_start(out=out_dram[:, :], in_=osb[:])

    return nc


_LAST_EXEC_NS = None


def _run(inputs, trace=False):
    global _LAST_EXEC_NS
    from concourse.bass_utils import run_bass_kernel_spmd

    cfg, shared, per_core = preprocess(**inputs)
    nc = bacc.Bacc("TRN2", target_bir_lowering=False, debug=False,
                   num_devices=M)
    build(nc, cfg)
    nc.compile()

    in_maps = []
    for k in range(M):
        m = dict(shared)
        m.update(per_core[k])
        in_maps.append({k2: np.ascontiguousarray(v) for k2, v in m.items()})

    res = run_bass_kernel_spmd(nc, in_maps, list(range(M)), trace=trace)
    _LAST_EXEC_NS = res.exec_time_ns

    out = np.zeros((N, CLS), np.float32)
    for k in range(M):
        o = res.results[k]["out"]  # [CLS, NPC_PAD]
        out[k * NPC:(k + 1) * NPC] = o[:CLS, :NPC].T
    return out


def kernel(**inputs):
    return _run(inputs, trace=False)



# revision 8
# speedup vs baseline: 1.6066x; 1.6066x over previous
"""DeepGAT (4-layer GAT + BN + residual + MLP head) on 8 Trainium2 cores.

v2 design. Nodes are dst-partitioned across 8 cores (1250/core). Per layer
each core assembles row-major per-node records [h bf16 128 | alpha_src 8 |
alpha_dst 8] for its own nodes; an AllGather concatenates them into the full
[N, 144] table in DRAM. Edges (dst-sorted, chunked by dst block of 128) fetch
their src record with one dma_gather per dst block. Per-edge alpha_dst is
reconstructed with a one-hot matmul (no second gather). Segment softmax is
normalized after aggregation. The per-head mean + GAT weight matmul is applied
on-device after aggregation via 8 PE transposes + 8 accumulating matmuls.

Pad edge slots gather a poison row (index N) whose alpha_src is -3e4 so their
exp() vanishes; no per-edge bias pass is needed.
"""

import numpy as np

import concourse.bass as bass
import concourse.bacc as bacc
import concourse.mybir as mybir
from concourse.tile import TileContext
from concourse.tile_rust import add_dep_helper

FP32 = mybir.dt.float32
BF16 = mybir.dt.bfloat16
I16 = mybir.dt.int16
AF = mybir.ActivationFunctionType
OP = mybir.AluOpType

ALPHA = 0.1
BN_EPS = 1e-5
NEG_SLOPE = 0.2
NEG_BIG = -30000.0
HID = 128

N, E, IN, H, L, CLS, M = 10000, 160000, 512, 8, 4, 2, 8
NPC = N // M                      # 1250
NPC_PAD = -(-NPC // 128) * 128    # 1280
NBLK = NPC_PAD // 128             # 10
HC = H * HID                      # 1024
RW = HID + 2 * H                  # 144  (h | as | ad)
RWG = 256                         # table row stride (512B, gather granule)
QD = HID // 2
KC = IN // 128

# serialize gathers (Q7 descriptor gen) against DVE work of previous block
SERIALIZE_GATHER = True


class Cfg:
    def __init__(self, chunks_per_block):
        self.chunks_per_block = list(chunks_per_block)
        self.CH = sum(self.chunks_per_block)
        self.TOTE = 128 * self.CH
        self.CBMAX = max(self.chunks_per_block)


def _pack_idx16(idx):
    """Pack int16 indices for dma_gather: idx i at [i%16, i//16], replicated
    to 128 partitions."""
    idx = np.asarray(idx, np.int64)
    n = len(idx)
    assert n % 16 == 0
    a = idx.astype(np.int16).reshape(n // 16, 16).T  # [16, n//16]
    return np.tile(a, (8, 1)).copy()


def preprocess(x, edge_index, Wp, bp, Wl, att_src, att_dst, bl, gamma, beta,
               W1, b1, W2, b2):
    x = np.asarray(x, np.float32)
    src = np.concatenate([np.asarray(edge_index[0]), np.arange(N)]).astype(np.int64)
    dst = np.concatenate([np.asarray(edge_index[1]), np.arange(N)]).astype(np.int64)

    per_core = []
    for k in range(M):
        m = (dst // NPC) == k
        s_k, d_k = src[m], dst[m] - k * NPC
        order = np.argsort(d_k, kind="stable")
        per_core.append((s_k[order], d_k[order]))

    counts = np.zeros((M, NBLK), np.int64)
    for k in range(M):
        _, d_k = per_core[k]
        b = d_k // 128
        for bb in range(NBLK):
            counts[k, bb] = int((b == bb).sum())
    chunks_per_block = [max(1, int(np.ceil(counts[:, bb].max() / 128)))
                        for bb in range(NBLK)]
    cfg = Cfg(chunks_per_block)

    per_core_inputs = []
    for k in range(M):
        s_k, d_k = per_core[k]
        b_k = d_k // 128
        srcidx = np.full(cfg.TOTE, N, np.int64)  # pad slots -> poison row
        ohv = np.zeros((128, cfg.CH, 128), np.float32)
        ohT = np.zeros((128, cfg.CH, 128), np.float32)
        coff = 0
        for bb in range(NBLK):
            sel = b_k == bb
            cnt = int(sel.sum())
            cb = cfg.chunks_per_block[bb]
            assert cnt <= 128 * cb
            t = coff * 128 + np.arange(cnt)
            srcidx[t] = s_k[sel]
            dloc = d_k[sel] - 128 * bb
            ohv[t % 128, t // 128, dloc] = 1.0
            ohT[dloc, t // 128, t % 128] = 1.0
            coff += cb
        assert coff == cfg.CH

        xT_own = np.zeros((IN, NPC_PAD), np.float32)
        xT_own[:, :NPC] = x[k * NPC:(k + 1) * NPC].T

        per_core_inputs.append({
            "srcidx": _pack_idx16(srcidx),
            "oh_in": _bf16(ohv),
            "ohT_in": _bf16(ohT),
            "xT_own": _bf16(xT_own),
        })

    # weight folding
    Wl = np.asarray(Wl, np.float32)          # [L, HID, HC]
    a_s = np.asarray(att_src, np.float32)    # [L, H, HID]
    a_d = np.asarray(att_dst, np.float32)
    wasad = np.zeros((L, HID, 2 * H), np.float32)
    for i in range(L):
        w3 = Wl[i].reshape(HID, H, HID)
        wasad[i, :, :H] = np.einsum("khc,hc->kh", w3, a_s[i])
        wasad[i, :, H:] = np.einsum("khc,hc->kh", w3, a_d[i])

    bn_inv = 1.0 / np.sqrt(1.0 + BN_EPS)
    gamma = np.asarray(gamma, np.float32)
    beta = np.asarray(beta, np.float32)
    bl = np.asarray(bl, np.float32)
    s_aff = ((1.0 - ALPHA) * gamma * bn_inv / H).T.copy()            # [HID, L]
    t_aff = ((1.0 - ALPHA) * (gamma * bn_inv * bl + beta)).T.copy()  # [HID, L]

    poison = np.zeros((1, RWG), np.float32)
    poison[0, HID:HID + H] = NEG_BIG

    shared = {
        "Wp": _bf16(np.asarray(Wp, np.float32)),
        "bp": np.asarray(bp, np.float32)[:, None].copy(),
        "Wl_in": _bf16(Wl),
        "wasad": _bf16(wasad),
        "s_aff": s_aff, "t_aff": t_aff,
        "W1": _bf16(np.asarray(W1, np.float32)),
        "b1": np.asarray(b1, np.float32)[:, None].copy(),
        "W2": _bf16(np.asarray(W2, np.float32)),
        "b2": np.asarray(b2, np.float32)[:, None].copy(),
        "ident": _bf16(np.eye(128, dtype=np.float32)),
        "poison": _bf16(poison),
    }
    return cfg, shared, per_core_inputs


def _bf16(a):
    import ml_dtypes
    return np.asarray(a, np.float32).astype(ml_dtypes.bfloat16)


def _elu(nc, p, out_ap, z_ap, shape, tg):
    """out = elu(z) = relu(z) + exp(min(z,0)) - 1, z in SBUF f32."""
    P, F = shape
    mn = p.tile([P, F], FP32, tag=f"elu_mn_{tg}")
    ex = p.tile([P, F], FP32, tag=f"elu_ex_{tg}")
    rl = p.tile([P, F], FP32, tag=f"elu_rl_{tg}")
    nc.vector.tensor_scalar_min(out=mn[:], in0=z_ap, scalar1=0.0)
    nc.scalar.activation(out=ex[:], in_=mn[:], func=AF.Exp)
    nc.vector.tensor_scalar_max(out=rl[:], in0=z_ap, scalar1=0.0)
    nc.vector.tensor_tensor(out=rl[:], in0=rl[:], in1=ex[:], op=OP.add)
    nc.vector.tensor_scalar_sub(out=out_ap, in0=rl[:], scalar1=1.0)


def build(nc, cfg):
    CH, CBMAX = cfg.CH, cfg.CBMAX

    # ---------------- I/O ----------------
    srcidx_in = nc.dram_tensor("srcidx", [128, cfg.TOTE // 16], I16, kind="ExternalInput")
    oh_in = nc.dram_tensor("oh_in", [128, CH, 128], BF16, kind="ExternalInput")
    ohT_in = nc.dram_tensor("ohT_in", [128, CH, 128], BF16, kind="ExternalInput")
    xT_in = nc.dram_tensor("xT_own", [IN, NPC_PAD], BF16, kind="ExternalInput")
    Wp_in = nc.dram_tensor("Wp", [IN, HID], BF16, kind="ExternalInput")
    bp_in = nc.dram_tensor("bp", [HID, 1], FP32, kind="ExternalInput")
    Wl_in = nc.dram_tensor("Wl_in", [L, HID, HC], BF16, kind="ExternalInput")
    wasad_in = nc.dram_tensor("wasad", [L, HID, 2 * H], BF16, kind="ExternalInput")
    s_aff_in = nc.dram_tensor("s_aff", [HID, L], FP32, kind="ExternalInput")
    t_aff_in = nc.dram_tensor("t_aff", [HID, L], FP32, kind="ExternalInput")
    W1_in = nc.dram_tensor("W1", [HID, QD], BF16, kind="ExternalInput")
    b1_in = nc.dram_tensor("b1", [QD, 1], FP32, kind="ExternalInput")
    W2_in = nc.dram_tensor("W2", [QD, CLS], BF16, kind="ExternalInput")
    b2_in = nc.dram_tensor("b2", [CLS, 1], FP32, kind="ExternalInput")
    ident_in = nc.dram_tensor("ident", [128, 128], BF16, kind="ExternalInput")
    poison_in = nc.dram_tensor("poison", [1, RWG], BF16, kind="ExternalInput")
    out_dram = nc.dram_tensor("out", [CLS, NPC_PAD], FP32, kind="ExternalOutput")

    agout = nc.dram_tensor("tbl", [N + 16, RWG], BF16,
                           addr_space="Shared" if M > 4 else "Local")

    with TileContext(nc) as tc:
        with (
            tc.tile_pool(name="const", bufs=1) as cpool,
            tc.tile_pool(name="state", bufs=1) as hpool,
            tc.tile_pool(name="gath", bufs=2) as gpool,
            tc.tile_pool(name="edge", bufs=2) as epool,
            tc.tile_pool(name="msgp", bufs=1) as mpool,
            tc.tile_pool(name="blk", bufs=2) as bpool,
            tc.tile_pool(name="wide", bufs=1) as wpool,
            tc.tile_pool(name="dram", bufs=2, space="DRAM") as dpool,
            tc.tile_pool(name="psA", bufs=1, space="PSUM") as psA,
            tc.tile_pool(name="psS", bufs=1, space="PSUM") as psS,
            tc.tile_pool(name="psM", bufs=2, space="PSUM") as psM,
            tc.tile_pool(name="psD", bufs=2, space="PSUM") as psD,
        ):
            _regs = {}

            def nreg(v):
                if v not in _regs:
                    _regs[v] = nc.gpsimd.to_reg(v)
                return _regs[v]

            # ---------------- constants ----------------
            srcidx_sb = cpool.tile([128, cfg.TOTE // 16], I16)
            nc.sync.dma_start(out=srcidx_sb[:], in_=srcidx_in[:, :])
            ident_sb = cpool.tile([128, 128], BF16)
            nc.sync.dma_start(out=ident_sb[:], in_=ident_in[:, :])
            s_aff = cpool.tile([128, L], FP32)
            nc.sync.dma_start(out=s_aff[:], in_=s_aff_in[:, :])
            t_aff = cpool.tile([128, L], FP32)
            nc.sync.dma_start(out=t_aff[:], in_=t_aff_in[:, :])
            W1_sb = cpool.tile([128, QD], BF16)
            nc.sync.dma_start(out=W1_sb[:], in_=W1_in[:, :])
            b1_sb = cpool.tile([QD, 1], FP32)
            nc.sync.dma_start(out=b1_sb[:], in_=b1_in[:, :])
            W2_sb = cpool.tile([QD, CLS], BF16)
            nc.sync.dma_start(out=W2_sb[:], in_=W2_in[:, :])
            b2_sb = cpool.tile([CLS, 1], FP32)
            nc.sync.dma_start(out=b2_sb[:], in_=b2_in[:, :])
            bp_sb = cpool.tile([HID, 1], FP32)
            nc.sync.dma_start(out=bp_sb[:], in_=bp_in[:, :])
            wasad_sb = cpool.tile([128, L, 2 * H], BF16)
            for i in range(L):
                nc.sync.dma_start(out=wasad_sb[:, i, :], in_=wasad_in[i, :, :])
            Wl_sb = cpool.tile([128, L, HC], BF16)
            for i in range(L):
                nc.sync.dma_start(out=Wl_sb[:, i, :], in_=Wl_in[i, :, :])
            Wp_sb = cpool.tile([128, KC, HID], BF16)
            for kc in range(KC):
                nc.sync.dma_start(out=Wp_sb[:, kc, :],
                                  in_=Wp_in[kc * 128:(kc + 1) * 128, :])

            # poison row for pad edge slots (persists across layers)
            pw = nc.sync.dma_start(out=agout[N:N + 1, :], in_=poison_in[:, :])

            h_own = [hpool.tile([128, NPC_PAD], FP32, tag=f"h_own{i}",
                                name=f"h_own{i}") for i in range(2)]
            h8 = hpool.tile([128, NPC_PAD], BF16, tag="h8")
            rows = hpool.tile([128, NBLK, RW], BF16, tag="rows")
            asadT8 = hpool.tile([16, NPC_PAD], BF16, tag="asadT8")

            pieces_n = [(j0, min(j0 + 512, NPC_PAD))
                        for j0 in range(0, NPC_PAD, 512)]

            # ------- h0 = elu(x @ Wp + bp) on own nodes -------
            with tc.tile_pool(name="x0", bufs=2) as x0pool:
                z0 = wpool.tile([128, NPC_PAD], FP32, tag="zw")
                for j0, j1 in pieces_n:
                    ps = psS.tile([128, 512], FP32, tag="mm512", name=f"h0ps{j0}")
                    for kc in range(KC):
                        xt = x0pool.tile([128, 512], BF16, tag="xT",
                                         name=f"xT{j0}_{kc}")
                        nc.sync.dma_start(
                            out=xt[:, 0:j1 - j0],
                            in_=xT_in[kc * 128:(kc + 1) * 128, j0:j1])
                        nc.tensor.matmul(out=ps[:, 0:j1 - j0],
                                         lhsT=Wp_sb[:, kc, :],
                                         rhs=xt[:, 0:j1 - j0],
                                         start=(kc == 0), stop=(kc == KC - 1))
                    nc.scalar.activation(out=z0[:, j0:j1], in_=ps[:, 0:j1 - j0],
                                         func=AF.Identity,
                                         bias=bp_sb[:, :1], scale=1.0)
                _elu(nc, wpool, h_own[0][:], z0[:], (128, NPC_PAD), "w")
                if NPC_PAD > NPC:
                    nc.vector.memset(h_own[0][:, NPC:], 0.0)

            # ---------------- layers ----------------
            for li in range(L):
                hprev = h_own[li % 2]
                hnew = h_own[(li + 1) % 2]

                # --- prep: rows = [h | as | ad] for own nodes ---
                nc.vector.tensor_copy(out=h8[:], in_=hprev[:])
                for j0, j1 in pieces_n:
                    ps = psS.tile([128, 512], FP32, tag="mm512",
                                  name=f"aps{li}_{j0}")
                    nc.tensor.matmul(out=ps[0:16, 0:j1 - j0],
                                     lhsT=wasad_sb[:, li, :],
                                     rhs=h8[:, j0:j1], start=True, stop=True)
                    nc.scalar.activation(out=asadT8[:, j0:j1],
                                         in_=ps[0:16, 0:j1 - j0],
                                         func=AF.Copy)
                bounce = dpool.tile([NPC, RWG], BF16, tag="bounce")
                bwrites = []
                for bb in range(NBLK):
                    s = bb * 128
                    ncols = min(128, NPC - s)
                    tp = psM.tile([128, 128], BF16, tag="tmini", name=f"tp{bb}")
                    nc.tensor.transpose(out=tp[:], in_=h8[:, s:s + 128],
                                        identity=ident_sb[:])
                    nc.scalar.activation(out=rows[:, bb, 0:HID], in_=tp[:],
                                         func=AF.Copy)
                    ta = psM.tile([128, 128], BF16, tag="tmini", name=f"ta{bb}")
                    nc.tensor.transpose(out=ta[:, 0:16], in_=asadT8[:, s:s + 128],
                                        identity=ident_sb[0:16, 0:16])
                    nc.scalar.activation(out=rows[:, bb, HID:RW],
                                         in_=ta[:, 0:16], func=AF.Copy)
                    w_ = nc.sync.dma_start(out=bounce[s:s + ncols, 0:RW],
                                           in_=rows[0:ncols, bb, :])
                    bwrites.append(w_)

                cc = nc.gpsimd.collective_compute(
                    "AllGather", OP.bypass,
                    replica_groups=[list(range(M))],
                    ins=[bounce[:]], outs=[agout[0:N, :]],
                )

                # --- edge phase ---
                prev_msg = None
                coff = 0
                for bb in range(NBLK):
                    cb = cfg.chunks_per_block[bb]
                    ohsb = gpool.tile([128, CBMAX, 128], BF16, tag="ohsb")
                    nc.sync.dma_start(out=ohsb[:, 0:cb, :],
                                      in_=oh_in[:, coff:coff + cb, :])
                    ohTsb = gpool.tile([128, CBMAX, 128], BF16, tag="ohTsb")
                    nc.sync.dma_start(out=ohTsb[:, 0:cb, :],
                                      in_=ohT_in[:, coff:coff + cb, :])

                    g = gpool.tile([128, CBMAX, RWG], BF16, tag="g")
                    g_ = nc.gpsimd.dma_gather(
                        out_ap=g[:, 0:cb, :], in_ap=agout[:, :],
                        idxs_ap=srcidx_sb[:, coff * 8:(coff + cb) * 8],
                        num_idxs=128 * cb, num_idxs_reg=nreg(128 * cb),
                        elem_size=RWG, single_packet=False)
                    add_dep_helper(g_.ins, cc.ins, True, "cc->gather")
                    add_dep_helper(g_.ins, pw.ins, True, "poison->gather")
                    if SERIALIZE_GATHER and prev_msg is not None:
                        add_dep_helper(g_.ins, prev_msg.ins, True, "dve->gather")

                    # ad_e: one matmul per chunk into PSUM [e, 8] slices
                    dn = psD.tile([128, 8 + 8 * CBMAX + 128], FP32, tag="dnad")
                    adps = dn[:, 8:8 + 8 * CBMAX]
                    for c in range(cb):
                        nc.tensor.matmul(out=adps[:, c * 8:(c + 1) * 8],
                                         lhsT=ohTsb[:, c, :],
                                         rhs=rows[:, bb, HID + H:RW],
                                         start=True, stop=True,
                                         skip_group_check=True)

                    # pe = exp(lrelu(as + ad)); pad slots carry as=-3e4
                    sv = epool.tile([128, CBMAX, H], FP32, tag="sv")
                    nc.vector.tensor_tensor(
                        out=sv[:, 0:cb, :], in0=g[:, 0:cb, HID:HID + H],
                        in1=adps[:, 0:8 * cb].rearrange("p (c h) -> p c h", h=H),
                        op=OP.add)
                    lr = epool.tile([128, CBMAX, H], FP32, tag="lr")
                    nc.vector.tensor_scalar_mul(out=lr[:, 0:cb, :],
                                                in0=sv[:, 0:cb, :],
                                                scalar1=NEG_SLOPE)
                    nc.vector.tensor_tensor(out=lr[:, 0:cb, :],
                                            in0=sv[:, 0:cb, :],
                                            in1=lr[:, 0:cb, :], op=OP.max)
                    pe = epool.tile([128, CBMAX, H, 1], BF16, tag="pe")
                    nc.scalar.activation(
                        out=pe[:, 0:cb, :, :].rearrange("p c h o -> p (c h o)"),
                        in_=lr[:, 0:cb, :].rearrange("p c h -> p (c h)"),
                        func=AF.Exp)

                    # msg[e, c, h, :] = h_row[e, c, :] * pe[e, c, h]
                    msg = mpool.tile([128, CBMAX, H, HID], BF16, tag="msg")
                    mop = nc.vector.tensor_tensor(
                        out=msg[:, 0:cb, :, :],
                        in0=g[:, 0:cb, 0:HID].unsqueeze(2).to_broadcast(
                            [128, cb, H, HID]),
                        in1=pe[:, 0:cb, :, :].to_broadcast([128, cb, H, HID]),
                        op=OP.mult)
                    prev_msg = mop

                    # aggregation
                    agg = psA.tile([128, HC], FP32, tag="agg")
                    den = dn[:, 0:8]
                    for c in range(cb):
                        first, last = c == 0, c == cb - 1
                        nc.tensor.matmul(out=agg[:, 0:512], lhsT=ohsb[:, c, :],
                                         rhs=msg[:, c, :, :].rearrange(
                                             "p h f -> p (h f)")[:, 0:512],
                                         start=first, stop=last,
                                         skip_group_check=True)
                        nc.tensor.matmul(out=agg[:, 512:HC], lhsT=ohsb[:, c, :],
                                         rhs=msg[:, c, :, :].rearrange(
                                             "p h f -> p (h f)")[:, 512:HC],
                                         start=first, stop=last,
                                         skip_group_check=True)
                        nc.tensor.matmul(out=den, lhsT=ohsb[:, c, :],
                                         rhs=pe[:, c, :, :].rearrange(
                                             "p h o -> p (h o)"),
                                         start=first, stop=last,
                                         skip_group_check=True)

                    # --- block epilogue ---
                    rec = bpool.tile([128, H], FP32, tag="rec")
                    nc.vector.tensor_scalar_max(out=rec[:], in0=den,
                                                scalar1=1e-20)
                    nc.vector.reciprocal(out=rec[:], in_=rec[:])
                    aggsb = bpool.tile([128, H, HID], BF16, tag="aggsb")
                    nc.vector.tensor_tensor(
                        out=aggsb[:],
                        in0=agg[:].rearrange("p (h f) -> p h f", h=H),
                        in1=rec[:].unsqueeze(2).to_broadcast([128, H, HID]),
                        op=OP.mult)
                    Tsb = bpool.tile([128, H, HID], BF16, tag="Tsb")
                    for hh in range(H):
                        tmini = psM.tile([128, 128], BF16, tag="tmini",
                                         name=f"tm{hh}")
                        nc.tensor.transpose(out=tmini[:], in_=aggsb[:, hh, :],
                                            identity=ident_sb[:])
                        nc.scalar.activation(out=Tsb[:, hh, :], in_=tmini[:],
                                             func=AF.Copy)
                    zpre = dn[:, 8 + 8 * CBMAX:]
                    for hh in range(H):
                        nc.tensor.matmul(
                            out=zpre,
                            lhsT=Wl_sb[:, li, hh * HID:(hh + 1) * HID],
                            rhs=Tsb[:, hh, :],
                            start=(hh == 0), stop=(hh == H - 1),
                            skip_group_check=True)
                    z1 = bpool.tile([128, 128], FP32, tag="z1")
                    nc.scalar.activation(out=z1[:], in_=zpre,
                                         func=AF.Identity,
                                         bias=t_aff[:, li:li + 1],
                                         scale=s_aff[:, li:li + 1])
                    z2 = bpool.tile([128, 128], FP32, tag="z2")
                    nc.vector.tensor_scalar_mul(
                        out=z2[:], in0=hprev[:, bb * 128:(bb + 1) * 128],
                        scalar1=ALPHA)
                    nc.vector.tensor_tensor(out=z1[:], in0=z1[:], in1=z2[:],
                                            op=OP.add)
                    _elu(nc, bpool, hnew[:, bb * 128:(bb + 1) * 128], z1[:],
                         (128, 128), "n")
                    coff += cb

            # ---------------- classifier ----------------
            hfin = h_own[L % 2]
            nc.vector.tensor_copy(out=h8[:], in_=hfin[:])
            zc = wpool.tile([QD, NPC_PAD], FP32, tag="zc")
            for j0, j1 in pieces_n:
                ps = psS.tile([128, 512], FP32, tag="mm512", name=f"clsps{j0}")
                nc.tensor.matmul(out=ps[0:QD, 0:j1 - j0], lhsT=W1_sb[:],
                                 rhs=h8[:, j0:j1], start=True, stop=True)
                nc.scalar.activation(out=zc[:, j0:j1], in_=ps[0:QD, 0:j1 - j0],
                                     func=AF.Identity,
                                     bias=b1_sb[:, :1], scale=1.0)
            hidsb = wpool.tile([QD, NPC_PAD], FP32, tag="hidsb")
            _elu(nc, wpool, hidsb[:], zc[:], (QD, NPC_PAD), "c")
            hid8 = wpool.tile([QD, NPC_PAD], BF16, tag="hid8")
            nc.vector.tensor_copy(out=hid8[:], in_=hidsb[:])
            osb = wpool.tile([CLS, NPC_PAD], FP32, tag="osb")
            for j0, j1 in pieces_n:
                ps = psS.tile([128, 512], FP32, tag="mm512", name=f"ops{j0}")
                nc.tensor.matmul(out=ps[0:CLS, 0:j1 - j0], lhsT=W2_sb[:],
                                 rhs=hid8[:, j0:j1], start=True, stop=True)
                nc.scalar.activation(out=osb[:, j0:j1], in_=ps[0:CLS, 0:j1 - j0],
                                     func=AF.Identity,
                                     bias=b2_sb[:, :1], scale=1.0)
            nc.sync.dma_start(out=out_dram[:, :], in_=osb[:])

    return nc


_LAST_EXEC_NS = None


def _run(inputs, trace=False):
    global _LAST_EXEC_NS
    from concourse.bass_utils import run_bass_kernel_spmd

    cfg, shared, per_core = preprocess(**inputs)
    nc = bacc.Bacc("TRN2", target_bir_lowering=False, debug=False,
                   num_devices=M)
    build(nc, cfg)
    nc.compile()

    in_maps = []
    for k in range(M):
        m = dict(shared)
        m.update(per_core[k])
        in_maps.append({k2: np.ascontiguousarray(v) for k2, v in m.items()})

    res = run_bass_kernel_spmd(nc, in_maps, list(range(M)), trace=trace)
    _LAST_EXEC_NS = res.exec_time_ns

    out = np.zeros((N, CLS), np.float32)
    for k in range(M):
        o = np.asarray(res.results[k]["out"], np.float32)
        out[k * NPC:(k + 1) * NPC] = o[:CLS, :NPC].T
    return out


def kernel(**inputs):
    return _run(inputs, trace=False)
